# revision 1
# baseline (speedup 1.0000x reference)
"""MicroStepDecoder TRN2 kernel (v2: bf16 everywhere, pipelined attention).

Math (equivalent to reference via causality/KV-cache):
  gather N=2048 rows -> h0 [N, D]; 5 decode steps of one llama layer,
  step t attends over cached K/V of steps 0..t; output[n, t] = h after step t.

Device strategy: data-parallel over 8 cores, 256 rows/core (2 row-tiles of 128).
Residual h kept natural [rows, D] fp32. All matmuls bf16 (1 cyc/row): lhsT =
transposed activations (PE transposes), rhs = streamed bf16 weight chunks.
RoPE / ln scales / 1/sqrt(hd) folded into weights on host.

Attention entirely on-chip in bf16 (2x DVE mode), split into two head-halves
(kv groups 0-3 / 4-7) so the DVE score/accumulate work of one half overlaps
the PE matmuls of the other; the e_j*V_j broadcast product (stride-0 AP, 1x
on DVE) is offloaded to GPSIMD. QKV chunk order is K,V,Q so attention starts
as early as possible; Q projection is skipped entirely at step 0 (single-key
softmax == V). Norms: DVE tensor_tensor_reduce (sum of squares) + ACT Rsqrt.
"""
import numpy as np
import ml_dtypes

import concourse.bass as bass
import concourse.bacc as bacc
import concourse.tile as tile
import concourse.mybir as mybir
from concourse.masks import make_identity
from concourse.bass_utils import run_bass_kernel_spmd

F32 = mybir.dt.float32
BF16 = mybir.dt.bfloat16
AX = mybir.AxisListType
ALU = mybir.AluOpType
ACTF = mybir.ActivationFunctionType

D = 2048
DFF = 8192
HEADS = 32
KVH = 8
HD = 64
REP = HEADS // KVH
STEPS = 5
NCORES = 8
R = 256            # rows per core
RT = 2             # row tiles per core
KT = D // 128      # 16
EPS = 1e-6
THETA = 1e4
NP_W = ml_dtypes.bfloat16
SIM_SAFE = False   # replace Silu with Sigmoid*x for CoreSim runs
GP_ATTN = True     # offload e_j*V_j broadcast product to GPSIMD
USE_TTR = False    # norm sum-of-squares via DVE tensor_tensor_reduce
                   # (tensor_tensor_reduce crashes the device on this stack)
NORM_FUSE = True   # compute norm sum-of-squares per 512-chunk right after each
                   # residual drain so it overlaps the remaining matmuls
SPLIT_O = True     # interleave O-proj accumulation with attention half B
ATTN_BF16 = True   # attention tensors in bf16 (else fp32)

_CACHE = {}


# ---------------------------------------------------------------- device views
def _q4h(ap):   # [128, 1024] -> [128, kv4, rep, hd]
    return ap.rearrange("p (kv r d) -> p kv r d", kv=KVH // 2, r=REP)


def _kv4h(ap):  # [128, 256] -> [128, kv4, rep(bc), hd]
    a3 = ap.rearrange("p (kv d) -> p kv d", kv=KVH // 2)
    return a3[:, :, None, :].broadcast_to((128, KVH // 2, REP, HD))


def _hb4h(ap):  # [128, 16] -> [128, kv4, rep, hd(bc)]
    a3 = ap.rearrange("p (kv r) -> p kv r", kv=KVH // 2)
    return a3[:, :, :, None].broadcast_to((128, KVH // 2, REP, HD))


# ---------------------------------------------------------------- program
def _build_program():
    nc = bacc.Bacc("TRN2", target_bir_lowering=False, debug=False)

    h0_d = nc.dram_tensor("h0", [RT, 128, D], F32, kind="ExternalInput")
    qkv_d = nc.dram_tensor("wqkv", [STEPS, 24, 128, 2048], BF16, kind="ExternalInput")
    o_d = nc.dram_tensor("wo", [16, 128, 2048], BF16, kind="ExternalInput")
    g_d = nc.dram_tensor("wg", [32, 128, 4096], BF16, kind="ExternalInput")
    u_d = nc.dram_tensor("wu", [32, 128, 4096], BF16, kind="ExternalInput")
    d_d = nc.dram_tensor("wd", [32, 128, 4096], BF16, kind="ExternalInput")
    out_d = nc.dram_tensor("out", [STEPS, RT, 128, D], F32, kind="ExternalOutput")

    with tile.TileContext(nc) as tc:
        with (
            tc.tile_pool(name="per", bufs=1) as per,       # persistent
            tc.tile_pool(name="scr", bufs=4) as scr,       # [128, D] bf16 scratch
            tc.tile_pool(name="asc", bufs=3 if ATTN_BF16 else 2) as asc,
            tc.tile_pool(name="avp", bufs=8 if ATTN_BF16 else 4) as avp,
            tc.tile_pool(name="wts", bufs=6 if ATTN_BF16 else 3) as wts,
            tc.tile_pool(name="sm", bufs=12) as sm,        # small tiles
            tc.tile_pool(name="ps_mm", bufs=4, space=bass.MemorySpace.PSUM) as ps_mm,
            tc.tile_pool(name="ps_gu", bufs=2, space=bass.MemorySpace.PSUM) as ps_gu,
            tc.tile_pool(name="ps_tp", bufs=2, space=bass.MemorySpace.PSUM) as ps_tp,
        ):
            ident_b = per.tile([128, 128], BF16, tag="ident_b", name="ident_b")
            make_identity(nc, ident_b[:])
            ADT = BF16 if ATTN_BF16 else F32
            if ATTN_BF16:
                ident_a = ident_b
            else:
                ident_a = per.tile([128, 128], F32, tag="ident_f", name="ident_f")
                make_identity(nc, ident_a[:])
            eps_t = per.tile([128, 1], F32, tag="eps", name="eps")
            nc.vector.memset(eps_t[:], EPS)

            h = [per.tile([128, D], F32, tag=f"h{rt}", name=f"h{rt}") for rt in range(RT)]
            Q = [[per.tile([128, 1024], ADT, tag=f"q{rt}_{hf}", name=f"q{rt}_{hf}")
                  for hf in range(2)] for rt in range(RT)]
            Kc = [[per.tile([128, 512], ADT, tag=f"kc{t}_{rt}", name=f"kc{t}_{rt}")
                   for rt in range(RT)] for t in range(STEPS)]
            Vc = [[per.tile([128, 512], ADT, tag=f"vc{t}_{rt}", name=f"vc{t}_{rt}")
                   for rt in range(RT)] for t in range(STEPS)]
            oa = [[per.tile([128, 1024], ADT, tag=f"oa{rt}_{hf}", name=f"oa{rt}_{hf}")
                   for hf in range(2)] for rt in range(RT)]
            xnT = per.tile([128, KT, R], BF16, tag="xnT", name="xnT")
            oT = per.tile([128, KT, R], BF16, tag="oT", name="oT")
            x2T = per.tile([128, KT, R], BF16, tag="x2T", name="x2T")
            mT = per.tile([128, 32, R], BF16, tag="mT", name="mT")

            for rt in range(RT):
                nc.sync.dma_start(h[rt][:], h0_d[:][rt])

            def norm_tp(dst, parts=None):
                # rms-norm both row tiles + transpose into dst [128, KT, R].
                # parts: per-rt list of 4 [128,1] partial sum-of-squares tiles
                # (computed chunk-wise during the preceding drains).
                ssqs = []
                for rt in range(RT):
                    ssq = sm.tile([128, 1], F32, tag="ssq", name="ssq")
                    if parts is not None:
                        p01 = sm.tile([128, 1], F32, tag="p01", name="p01")
                        nc.vector.tensor_add(p01[:], parts[rt][0][:], parts[rt][1][:])
                        p23 = sm.tile([128, 1], F32, tag="p23", name="p23")
                        nc.vector.tensor_add(p23[:], parts[rt][2][:], parts[rt][3][:])
                        nc.vector.tensor_add(ssq[:], p01[:], p23[:])
                    else:
                        junk = scr.tile([128, D], BF16, tag="junk", name="junk")
                        nc.scalar.activation(junk[:], h[rt][:], func=ACTF.Square,
                                             accum_out=ssq[:])
                    ssqs.append(ssq)
                sds = []
                for rt in range(RT):
                    sd = sm.tile([128, 1], F32, tag="sd", name="sd")
                    nc.scalar.activation(sd[:], ssqs[rt][:], func=ACTF.Sqrt,
                                         scale=1.0 / D, bias=eps_t[:])
                    sds.append(sd)
                for rt in range(RT):
                    rstd = sm.tile([128, 1], F32, tag="rstd", name="rstd")
                    nc.vector.reciprocal(rstd[:], sds[rt][:])
                    x = scr.tile([128, D], BF16, tag="scr", name="xn")
                    nc.vector.tensor_scalar_mul(x[:], h[rt][:], rstd[:])
                    for k in range(KT):
                        tp = ps_tp.tile([128, 128], BF16, tag="tp", name="tp")
                        nc.tensor.transpose(
                            tp[:], x[:, k * 128:(k + 1) * 128], ident_b[:])
                        nc.vector.tensor_copy(
                            dst[:, k, rt * 128:(rt + 1) * 128], tp[:])

            def drain_add(rt, ch, pt, parts):
                # h[rt] chunk += psum; optionally compute the chunk's
                # sum-of-squares for the next norm while matmuls continue
                nc.vector.tensor_add(
                    h[rt][:, ch * 512:(ch + 1) * 512],
                    h[rt][:, ch * 512:(ch + 1) * 512], pt[:])
                if parts is not None:
                    junk = scr.tile([128, 512], BF16, tag="junk", name="jk5")
                    part = sm.tile([128, 1], F32, tag="sqp", name="sqp")
                    nc.scalar.activation(junk[:], h[rt][:, ch * 512:(ch + 1) * 512],
                                         func=ACTF.Square, accum_out=part[:])
                    parts[rt].append(part)

            dum = per.tile([128, 1], F32, tag="dum", name="dum")
            nc.vector.memset(dum[:], 1.0)
            dumo = per.tile([128, 1], F32, tag="dumo", name="dumo")

            def act_preload(func):
                # dummy activation so the ACT table load happens off the
                # critical path (during a matmul-heavy phase)
                nc.scalar.activation(dumo[:], dum[:], func=func)

            dn_parts = None
            for t in range(STEPS):
                # ---- norm1 + transpose -> xnT (bf16)
                norm_tp(xnT, dn_parts)
                if t >= 1:
                    act_preload(ACTF.Exp)

                # ---- QKV projections; ch order: K, V, Q0..Q3 (Q skipped at t=0)
                nch = 2 if t == 0 else 6
                for ch in range(nch):
                    pq = [ps_mm.tile([128, 512], F32, tag="mm", name=f"pq{_rt}")
                          for _rt in range(RT)]
                    for kg in range(4):
                        w = wts.tile([128, 4096], BF16, tag="w", name="w")
                        nc.sync.dma_start(w[:, :2048], qkv_d[:][t, ch * 4 + kg])
                        for i in range(4):
                            k = kg * 4 + i
                            for rt in range(RT):
                                nc.tensor.matmul(
                                    pq[rt][:],
                                    xnT[:, k, rt * 128:(rt + 1) * 128],
                                    w[:, i * 512:(i + 1) * 512],
                                    start=(k == 0), stop=(k == KT - 1))
                    for rt in range(RT):
                        if ch == 0:
                            nc.vector.tensor_copy(Kc[t][rt][:], pq[rt][:])
                        elif ch == 1:
                            nc.vector.tensor_copy(Vc[t][rt][:], pq[rt][:])
                        else:
                            hf, part = (ch - 2) // 2, (ch - 2) % 2
                            nc.vector.tensor_copy(
                                Q[rt][hf][:, part * 512:(part + 1) * 512], pq[rt][:])

                # ---- attention (two head-halves) + O projection, interleaved
                def attn(hf):
                    # Phase 1 (both row tiles): scores -> exp -> e_j*V_j, with
                    # the oa-accumulation deferred so the DVE FIFO never stalls
                    # behind a GPSIMD product. Phase 2: den, adds, normalize.
                    k0, k1 = hf * 256, (hf + 1) * 256
                    eng = nc.gpsimd if GP_ATTN else nc.vector
                    ejs = [[] for _ in range(RT)]
                    avs = [[] for _ in range(RT)]
                    for rt in range(RT):
                        oah = oa[rt][hf]
                        if t == 0:
                            nc.vector.tensor_copy(
                                _q4h(oah[:]), _kv4h(Vc[0][rt][:, k0:k1]))
                            continue
                        for j in range(t + 1):
                            ascr = asc.tile([128, 1024], ADT, tag="ascr",
                                            name="ascr")
                            nc.vector.tensor_tensor(
                                _q4h(ascr[:]), _q4h(Q[rt][hf][:]),
                                _kv4h(Kc[j][rt][:, k0:k1]), op=ALU.mult)
                            sc = sm.tile([128, 16], F32, tag="sc", name="sc")
                            nc.vector.tensor_reduce(
                                sc[:],
                                ascr[:].rearrange("p (h d) -> p h d", h=16),
                                axis=AX.X, op=ALU.add)
                            ej = sm.tile([128, 16], F32, tag="ej", name="ej")
                            nc.scalar.activation(ej[:], sc[:], func=ACTF.Exp)
                            ejs[rt].append(ej)
                            if j == 0:
                                eng.tensor_tensor(
                                    _q4h(oah[:]), _hb4h(ej[:]),
                                    _kv4h(Vc[j][rt][:, k0:k1]), op=ALU.mult)
                            else:
                                av = avp.tile([128, 1024], ADT, tag="av",
                                              name="av")
                                eng.tensor_tensor(
                                    _q4h(av[:]), _hb4h(ej[:]),
                                    _kv4h(Vc[j][rt][:, k0:k1]), op=ALU.mult)
                                avs[rt].append(av)
                    for rt in range(RT):
                        if t == 0:
                            continue
                        oah = oa[rt][hf]
                        den = sm.tile([128, 16], F32, tag="den", name="den")
                        nc.vector.tensor_add(den[:], ejs[rt][0][:], ejs[rt][1][:])
                        for ej in ejs[rt][2:]:
                            nc.vector.tensor_add(den[:], den[:], ej[:])
                        rec = sm.tile([128, 16], F32, tag="rec", name="rec")
                        nc.vector.reciprocal(rec[:], den[:])
                        for av in avs[rt]:
                            nc.vector.tensor_add(oah[:], oah[:], av[:])
                        nc.vector.tensor_tensor(
                            _q4h(oah[:]), _hb4h(rec[:]),
                            _q4h(oah[:]), op=ALU.mult)

                def o_tp(hf):
                    for kk in range(8):
                        k = hf * 8 + kk
                        for rt in range(RT):
                            tp = ps_tp.tile([128, 128], ADT, tag="tp", name="tp")
                            nc.tensor.transpose(
                                tp[:], oa[rt][hf][:, kk * 128:(kk + 1) * 128],
                                ident_a[:])
                            nc.vector.tensor_copy(
                                oT[:, k, rt * 128:(rt + 1) * 128], tp[:])

                def o_mms(po, chs, kgs):
                    for ch in chs:
                        for kg in kgs:
                            w = wts.tile([128, 4096], BF16, tag="w", name="w")
                            nc.sync.dma_start(w[:, :2048], o_d[:][ch * 4 + kg])
                            for i in range(4):
                                k = kg * 4 + i
                                for rt in range(RT):
                                    nc.tensor.matmul(
                                        po[ch][rt][:],
                                        oT[:, k, rt * 128:(rt + 1) * 128],
                                        w[:, i * 512:(i + 1) * 512],
                                        start=(k == 0), stop=(k == KT - 1))

                def o_drain(po, chs, parts):
                    for ch in chs:
                        for rt in range(RT):
                            drain_add(rt, ch, po[ch][rt], parts)

                o_parts = [[] for _ in range(RT)] if NORM_FUSE else None
                if SPLIT_O:
                    attn(0)
                    o_tp(0)
                    po01 = {ch: [ps_mm.tile([128, 512], F32, tag="mm", name=f"po{ch}{_rt}")
                                 for _rt in range(RT)] for ch in (0, 1)}
                    o_mms(po01, (0, 1), (0, 1))
                    attn(1)
                    o_tp(1)
                    o_mms(po01, (0, 1), (2, 3))
                    o_drain(po01, (0, 1), o_parts)
                    po23 = {ch: [ps_mm.tile([128, 512], F32, tag="mm", name=f"po{ch}{_rt}")
                                 for _rt in range(RT)] for ch in (2, 3)}
                    o_mms(po23, (2, 3), (0, 1, 2, 3))
                    o_drain(po23, (2, 3), o_parts)
                else:
                    attn(0)
                    o_tp(0)
                    attn(1)
                    o_tp(1)
                    for chp in ((0, 1), (2, 3)):
                        po = {ch: [ps_mm.tile([128, 512], F32, tag="mm",
                                              name=f"po{ch}{_rt}")
                                   for _rt in range(RT)] for ch in chp}
                        o_mms(po, chp, (0, 1, 2, 3))
                        o_drain(po, chp, o_parts)

                # ---- norm2 + transpose -> x2T (bf16)
                norm_tp(x2T, o_parts)
                act_preload(ACTF.Sigmoid if SIM_SAFE else ACTF.Silu)

                # ---- MLP in two ff halves: gate/up -> mT, then down
                dn_parts = ([[] for _ in range(RT)]
                            if (NORM_FUSE and t < STEPS - 1) else None)
                for half in range(2):
                    for pr in range(16 * half, 16 * (half + 1)):
                        wg = wts.tile([128, 4096], BF16, tag="w", name="wgt")
                        nc.sync.dma_start(wg[:], g_d[:][pr])
                        wu = wts.tile([128, 4096], BF16, tag="w", name="wut")
                        nc.sync.dma_start(wu[:], u_d[:][pr])
                        for mgi in range(2):
                            mloc = (pr * 2 + mgi) - 32 * half
                            pg = ps_gu.tile([128, R], F32, tag="gu", name="pg")
                            for k in range(KT):
                                c = (mgi * KT + k) * 128
                                nc.tensor.matmul(
                                    pg[:], wg[:, c:c + 128], x2T[:, k, :],
                                    start=(k == 0), stop=(k == KT - 1))
                            pu = ps_gu.tile([128, R], F32, tag="gu", name="pu")
                            for k in range(KT):
                                c = (mgi * KT + k) * 128
                                nc.tensor.matmul(
                                    pu[:], wu[:, c:c + 128], x2T[:, k, :],
                                    start=(k == 0), stop=(k == KT - 1))
                            sg = sm.tile([128, R], BF16, tag="sg", name="sg")
                            if SIM_SAFE:
                                # CoreSim lacks Silu; silu(x) = x * sigmoid(x)
                                nc.scalar.activation(sg[:], pg[:], func=ACTF.Sigmoid)
                                tmp = asc.tile([128, R], F32, tag="sgt", name="sgt")
                                nc.vector.tensor_tensor(
                                    tmp[:], sg[:], pg[:], op=ALU.mult)
                                nc.vector.tensor_tensor(
                                    mT[:, mloc, :], tmp[:], pu[:], op=ALU.mult)
                            else:
                                nc.scalar.activation(sg[:], pg[:], func=ACTF.Silu)
                                nc.vector.tensor_tensor(
                                    mT[:, mloc, :], sg[:], pu[:], op=ALU.mult)
                    for ch in range(4):
                        pd_ = [ps_mm.tile([128, 512], F32, tag="mm", name=f"pd{_rt}")
                               for _rt in range(RT)]
                        for kfg in range(4 * half, 4 * (half + 1)):
                            w = wts.tile([128, 4096], BF16, tag="w", name="wdt")
                            nc.sync.dma_start(w[:], d_d[:][ch * 8 + kfg])
                            for i in range(8):
                                kf = kfg * 8 + i
                                kfl = kf - 32 * half
                                for rt in range(RT):
                                    nc.tensor.matmul(
                                        pd_[rt][:],
                                        mT[:, kfl, rt * 128:(rt + 1) * 128],
                                        w[:, i * 512:(i + 1) * 512],
                                        start=(kf == 32 * half),
                                        stop=(kf == 32 * half + 31))
                        for rt in range(RT):
                            drain_add(rt, ch, pd_[rt],
                                      dn_parts if half == 1 else None)

                # ---- store step output
                for rt in range(RT):
                    nc.gpsimd.dma_start(out_d[:][t, rt], h[rt][:])

    nc.compile()
    return nc


# ---------------------------------------------------------------- host prep
def _rope_cs(t):
    inv = 1.0 / (THETA ** (np.arange(0, HD, 2, dtype=np.float64) / HD))
    emb = np.concatenate([t * inv, t * inv])
    return np.cos(emb), np.sin(emb)


def _rope_cols(w, t, nheads):
    # w: [D, nheads*HD] fp; returns rope'd version for position t
    w3 = w.reshape(D, nheads, HD)
    cos, sin = _rope_cs(t)
    wrot = np.concatenate([-w3[:, :, HD // 2:], w3[:, :, :HD // 2]], axis=2)
    return (w3 * cos[None, None, :] + wrot * sin[None, None, :]).reshape(D, nheads * HD)


def _pack_rhs(w, n_ch, n_kg):
    # w [K, n_ch*512]; chunks (ch, kg): [128, 4*512]; kg covers 4 k-tiles
    kt = w.shape[0] // 128
    A = w.reshape(n_kg, kt // n_kg, 128, n_ch, 512)
    return np.ascontiguousarray(A.transpose(3, 0, 2, 1, 4)).reshape(
        n_ch * n_kg, 128, (kt // n_kg) * 512)


def _pack_lhs_gu(w):
    # w [D, DFF] -> [32 pairs][128, (mgi 2, k 16, 128)]
    B = w.reshape(KT, 128, 32, 2, 128)
    return np.ascontiguousarray(B.transpose(2, 1, 3, 0, 4)).reshape(32, 128, 4096)


def _pack_rhs_dn(w):
    # w [DFF, D] -> chunks (ch 4, kfg 8): [128, (i 8, 512)]
    C = w.reshape(8, 8, 128, 4, 512)
    return np.ascontiguousarray(C.transpose(3, 0, 2, 1, 4)).reshape(32, 128, 4096)


def _gather_indices(comp_seq_lens, inst_lens):
    seqs = np.asarray(comp_seq_lens)
    insts = np.asarray(inst_lens)
    idx, off = [], 0
    for s, i in zip(seqs, insts):
        s, i = int(s), int(i)
        idx.append(np.arange(off + i - 1, off + s - 1))
        off += s
    return np.concatenate(idx)


def _prep_inputs(hidden_states, comp_seq_lens, inst_lens, w_q, w_k, w_v, w_o,
                 ln1_w, ln2_w, w_gate, w_up, w_down):
    idx = _gather_indices(comp_seq_lens, inst_lens)
    h0 = np.asarray(hidden_states, np.float32)[0, idx]          # [N, D]
    N = h0.shape[0]
    assert N == NCORES * R, f"expected {NCORES*R} rows, got {N}"

    ln1 = np.asarray(ln1_w, np.float64)
    ln2 = np.asarray(ln2_w, np.float64)
    wq_e = np.asarray(w_q, np.float64) * ln1[:, None] * (HD ** -0.5)
    wk_e = np.asarray(w_k, np.float64) * ln1[:, None]
    wv_e = np.asarray(w_v, np.float64) * ln1[:, None]
    wg_e = np.asarray(w_gate, np.float64) * ln2[:, None]
    wu_e = np.asarray(w_up, np.float64) * ln2[:, None]

    qkv_pack = np.empty((STEPS, 24, 128, 2048), NP_W)
    for t in range(STEPS):
        wq_t = _rope_cols(wq_e, t, HEADS)
        wk_t = _rope_cols(wk_e, t, KVH)
        qkv = np.concatenate([wk_t, wv_e, wq_t], axis=1)  # K, V, Q order
        qkv_pack[t] = _pack_rhs(qkv, 6, 4).astype(NP_W)

    weights = {
        "wqkv": qkv_pack,
        "wo": _pack_rhs(np.asarray(w_o, np.float64), 4, 4).astype(NP_W),
        "wg": _pack_lhs_gu(wg_e).astype(NP_W),
        "wu": _pack_lhs_gu(wu_e).astype(NP_W),
        "wd": _pack_rhs_dn(np.asarray(w_down, np.float64)).astype(NP_W),
    }
    h0_cores = h0.reshape(NCORES, RT, 128, D)
    return weights, h0_cores


def kernel(**inputs):
    weights, h0_cores = _prep_inputs(**inputs)

    if "nc" not in _CACHE:
        _CACHE["nc"] = _build_program()
    nc = _CACHE["nc"]

    in_maps = [dict(weights, h0=np.ascontiguousarray(h0_cores[c]))
               for c in range(NCORES)]
    res = run_bass_kernel_spmd(nc, in_maps, core_ids=list(range(NCORES)))
    _CACHE["last_results"] = res

    outs = []
    for c in range(NCORES):
        o = res.results[c]["out"]                  # [5, RT, 128, D]
        outs.append(o.reshape(STEPS, R, D).transpose(1, 0, 2))
    return np.concatenate(outs, axis=0)            # [N, 5, D]



# revision 4
# speedup vs baseline: 1.0205x; 1.0205x over previous
"""MicroStepDecoder TRN2 kernel (v3: row-tile staggered pipeline).

Math (equivalent to reference via causality/KV-cache):
  gather N=2048 rows -> h0 [N, D]; 5 decode steps of one llama layer,
  step t attends over cached K/V of steps 0..t; output[n, t] = h after step t.

Device strategy: data-parallel over 8 cores, 256 rows/core (2 row-tiles of
128).  All matmuls bf16; lhsT = transposed activations (PE transposes), rhs =
streamed bf16 weight chunks (gate/up: weights stationary, activations moving).
RoPE / ln scales / 1/sqrt(hd) folded into weights on host.

v3 structural change vs v2: the two row-tiles are software-pipelined so the
DVE/ACT/GPSIMD attention chain of one row-tile overlaps PE matmuls of the
other (QKV of rt1 hides attn(rt0); O-proj of rt0 hides attn(rt1)).  norm1 is
folded into the QKV psum copy-outs (ACT copy with per-partition rstd scale)
so xnT is a raw transpose of h and step boundaries have no serial
normalize->transpose chain.  QKV weight chunks are loaded once and reused by
both row-tiles; rt1's QKV copy-outs are deferred until after attn(rt0)'s exps
so the ACT queue never blocks the attention chain.
"""
import numpy as np
import ml_dtypes

import concourse.bass as bass
import concourse.bacc as bacc
import concourse.tile as tile
import concourse.mybir as mybir
from concourse.masks import make_identity
from concourse.bass_utils import run_bass_kernel_spmd

F32 = mybir.dt.float32
BF16 = mybir.dt.bfloat16
AX = mybir.AxisListType
ALU = mybir.AluOpType
ACTF = mybir.ActivationFunctionType

D = 2048
DFF = 8192
HEADS = 32
KVH = 8
HD = 64
REP = HEADS // KVH
STEPS = 5
NCORES = 8
R = 256            # rows per core
RT = 2             # row tiles per core
KT = D // 128      # 16
EPS = 1e-6
THETA = 1e4
NP_W = ml_dtypes.bfloat16
SIM_SAFE = False   # replace Silu with Sigmoid*x for CoreSim runs
GP_ATTN = True     # offload e_j*V_j broadcast product to GPSIMD

_CACHE = {}


# ---------------------------------------------------------------- device views
def _q4h(ap):   # [128, 1024] -> [128, kv4, rep, hd]
    return ap.rearrange("p (kv r d) -> p kv r d", kv=KVH // 2, r=REP)


def _kv4h(ap):  # [128, 256] -> [128, kv4, rep(bc), hd]
    a3 = ap.rearrange("p (kv d) -> p kv d", kv=KVH // 2)
    return a3[:, :, None, :].broadcast_to((128, KVH // 2, REP, HD))


def _hb4h(ap):  # [128, 16] -> [128, kv4, rep, hd(bc)]
    a3 = ap.rearrange("p (kv r) -> p kv r", kv=KVH // 2)
    return a3[:, :, :, None].broadcast_to((128, KVH // 2, REP, HD))


# ---------------------------------------------------------------- program
def _build_program():
    nc = bacc.Bacc("TRN2", target_bir_lowering=False, debug=False)

    h0_d = nc.dram_tensor("h0", [RT, 128, D], F32, kind="ExternalInput")
    qkv_d = nc.dram_tensor("wqkv", [STEPS, 24, 128, 2048], BF16, kind="ExternalInput")
    o_d = nc.dram_tensor("wo", [16, 128, 2048], BF16, kind="ExternalInput")
    g_d = nc.dram_tensor("wg", [32, 128, 4096], BF16, kind="ExternalInput")
    u_d = nc.dram_tensor("wu", [32, 128, 4096], BF16, kind="ExternalInput")
    d_d = nc.dram_tensor("wd", [32, 128, 4096], BF16, kind="ExternalInput")
    out_d = nc.dram_tensor("out", [STEPS, RT, 128, D], F32, kind="ExternalOutput")

    with tile.TileContext(nc) as tc:
        with (
            tc.tile_pool(name="per", bufs=1) as per,       # persistent
            tc.tile_pool(name="scr", bufs=4) as scr,       # [128, D] bf16 scratch
            tc.tile_pool(name="asc", bufs=3) as asc,
            tc.tile_pool(name="avp", bufs=8) as avp,
            tc.tile_pool(name="wts", bufs=6) as wts,
            tc.tile_pool(name="sm", bufs=12) as sm,        # small tiles
            tc.tile_pool(name="ps_mm", bufs=4, space=bass.MemorySpace.PSUM) as ps_mm,
            tc.tile_pool(name="ps_gu", bufs=2, space=bass.MemorySpace.PSUM) as ps_gu,
            tc.tile_pool(name="ps_tp", bufs=2, space=bass.MemorySpace.PSUM) as ps_tp,
        ):
            ident_b = per.tile([128, 128], BF16, tag="ident_b", name="ident_b")
            make_identity(nc, ident_b[:])
            ident_f = per.tile([128, 128], F32, tag="ident_f", name="ident_f")
            make_identity(nc, ident_f[:])
            ADT = BF16
            eps_t = per.tile([128, 1], F32, tag="eps", name="eps")
            nc.vector.memset(eps_t[:], EPS)

            h = [per.tile([128, D], F32, tag=f"h{rt}", name=f"h{rt}") for rt in range(RT)]
            Q = [[per.tile([128, 1024], ADT, tag=f"q{rt}_{hf}", name=f"q{rt}_{hf}")
                  for hf in range(2)] for rt in range(RT)]
            Kc = [[per.tile([128, 512], ADT, tag=f"kc{t}_{rt}", name=f"kc{t}_{rt}")
                   for rt in range(RT)] for t in range(STEPS)]
            Vc = [[per.tile([128, 512], ADT, tag=f"vc{t}_{rt}", name=f"vc{t}_{rt}")
                   for rt in range(RT)] for t in range(STEPS)]
            oa = [[per.tile([128, 1024], ADT, tag=f"oa{rt}_{hf}", name=f"oa{rt}_{hf}")
                   for hf in range(2)] for rt in range(RT)]
            # per-rt transposed activations (raw h, bf16) and attn output
            xnT = [per.tile([128, KT, 128], BF16, tag=f"xnT{rt}", name=f"xnT{rt}")
                   for rt in range(RT)]
            oT = [per.tile([128, KT, 128], BF16, tag=f"oT{rt}", name=f"oT{rt}")
                  for rt in range(RT)]
            x2T = per.tile([128, KT, R], BF16, tag="x2T", name="x2T")
            mT = per.tile([128, 32, R], BF16, tag="mT", name="mT")

            for rt in range(RT):
                nc.sync.dma_start(h[rt][:], h0_d[:][rt])

            dum = per.tile([128, 1], F32, tag="dum", name="dum")
            nc.vector.memset(dum[:], 1.0)
            dumo = per.tile([128, 1], F32, tag="dumo", name="dumo")

            def act_preload(func):
                # dummy activation so the ACT table load happens off the
                # critical path (during a matmul-heavy phase)
                nc.scalar.activation(dumo[:], dum[:], func=func)

            def rstd_of(rt, parts):
                # 1/rms of h[rt]; parts = 4 chunked sum-of-squares partials
                ssq = sm.tile([128, 1], F32, tag="ssq", name="ssq")
                if parts is not None:
                    p01 = sm.tile([128, 1], F32, tag="p01", name="p01")
                    nc.vector.tensor_add(p01[:], parts[rt][0][:], parts[rt][1][:])
                    p23 = sm.tile([128, 1], F32, tag="p23", name="p23")
                    nc.vector.tensor_add(p23[:], parts[rt][2][:], parts[rt][3][:])
                    nc.vector.tensor_add(ssq[:], p01[:], p23[:])
                else:
                    junk = scr.tile([128, D], BF16, tag="junk", name="junk")
                    nc.scalar.activation(junk[:], h[rt][:], func=ACTF.Square,
                                         accum_out=ssq[:])
                sd = sm.tile([128, 1], F32, tag="sd", name="sd")
                nc.scalar.activation(sd[:], ssq[:], func=ACTF.Sqrt,
                                     scale=1.0 / D, bias=eps_t[:])
                rstd = sm.tile([128, 1], F32, tag="rstd", name="rstd")
                nc.vector.reciprocal(rstd[:], sd[:])
                return rstd

            def tp_h(rt):
                # raw transpose of fp32 residual -> xnT[rt] (bf16)
                for k in range(KT):
                    tp = ps_tp.tile([128, 128], F32, tag="tp", name="tp")
                    nc.tensor.transpose(
                        tp[:], h[rt][:, k * 128:(k + 1) * 128], ident_f[:])
                    nc.vector.tensor_copy(xnT[rt][:, k, :], tp[:])

            def drain_add(rt, ch, pt, parts):
                # h[rt] chunk += psum; optionally compute the chunk's
                # sum-of-squares for the next norm while matmuls continue
                nc.vector.tensor_add(
                    h[rt][:, ch * 512:(ch + 1) * 512],
                    h[rt][:, ch * 512:(ch + 1) * 512], pt[:])
                if parts is not None:
                    junk = scr.tile([128, 512], BF16, tag="junk", name="jk5")
                    part = sm.tile([128, 1], F32, tag="sqp", name="sqp")
                    nc.scalar.activation(junk[:], h[rt][:, ch * 512:(ch + 1) * 512],
                                         func=ACTF.Square, accum_out=part[:])
                    parts[rt].append(part)

            dn_parts = None
            for t in range(STEPS):
                nch = 2 if t == 0 else 6
                rstds = {}
                pq_held = {}

                def qkv_rt(rt, defer_copies):
                    for ch in range(nch):
                        pool = ps_mm if ch < 4 else ps_gu
                        pq = pool.tile([128, 512], F32, tag="mm" if ch < 4 else "gu",
                                       name=f"pq{rt}")
                        for kg in range(4):
                            w = wts.tile([128, 4096], BF16, tag="w", name="wqkv")
                            nc.sync.dma_start(w[:, :2048], qkv_d[:][t, ch * 4 + kg])
                            for i in range(4):
                                k = kg * 4 + i
                                nc.tensor.matmul(
                                    pq[:], xnT[rt][:, k, :],
                                    w[:, i * 512:(i + 1) * 512],
                                    start=(k == 0), stop=(k == KT - 1))
                        if defer_copies:
                            pq_held[ch] = pq
                        else:
                            qkv_copy(rt, ch, pq)

                def qkv_copy(rt, ch, pq):
                    # psum -> SBUF on ACT, with the rms-norm rstd folded in
                    if ch == 0:
                        dst = Kc[t][rt][:]
                    elif ch == 1:
                        dst = Vc[t][rt][:]
                    else:
                        hf, part = (ch - 2) // 2, (ch - 2) % 2
                        dst = Q[rt][hf][:, part * 512:(part + 1) * 512]
                    nc.scalar.activation(dst, pq[:], func=ACTF.Copy,
                                         scale=rstds[rt][:])

                def attn_rt(rt):
                    eng = nc.gpsimd if GP_ATTN else nc.vector
                    if t == 0:
                        # single-key softmax == V (scaled by nothing; V copy)
                        for hf in range(2):
                            k0, k1 = hf * 256, (hf + 1) * 256
                            nc.vector.tensor_copy(
                                _q4h(oa[rt][hf][:]), _kv4h(Vc[0][rt][:, k0:k1]))
                        return
                    ejs = {0: [], 1: []}
                    avs = {0: [], 1: []}
                    for hf in range(2):
                        k0, k1 = hf * 256, (hf + 1) * 256
                        oah = oa[rt][hf]
                        for j in range(t + 1):
                            ascr = asc.tile([128, 1024], ADT, tag="ascr",
                                            name="ascr")
                            nc.vector.tensor_tensor(
                                _q4h(ascr[:]), _q4h(Q[rt][hf][:]),
                                _kv4h(Kc[j][rt][:, k0:k1]), op=ALU.mult)
                            sc = sm.tile([128, 16], F32, tag="sc", name="sc")
                            nc.vector.tensor_reduce(
                                sc[:],
                                ascr[:].rearrange("p (h d) -> p h d", h=16),
                                axis=AX.X, op=ALU.add)
                            ej = sm.tile([128, 16], F32, tag="ej", name="ej")
                            nc.scalar.activation(ej[:], sc[:], func=ACTF.Exp)
                            ejs[hf].append(ej)
                            if j == 0:
                                eng.tensor_tensor(
                                    _q4h(oah[:]), _hb4h(ej[:]),
                                    _kv4h(Vc[j][rt][:, k0:k1]), op=ALU.mult)
                            else:
                                av = avp.tile([128, 1024], ADT, tag="av",
                                              name="av")
                                eng.tensor_tensor(
                                    _q4h(av[:]), _hb4h(ej[:]),
                                    _kv4h(Vc[j][rt][:, k0:k1]), op=ALU.mult)
                                avs[hf].append(av)
                    for hf in range(2):
                        oah = oa[rt][hf]
                        den = sm.tile([128, 16], F32, tag="den", name="den")
                        nc.vector.tensor_add(den[:], ejs[hf][0][:], ejs[hf][1][:])
                        for ej in ejs[hf][2:]:
                            nc.vector.tensor_add(den[:], den[:], ej[:])
                        rec = sm.tile([128, 16], F32, tag="rec", name="rec")
                        nc.vector.reciprocal(rec[:], den[:])
                        for av in avs[hf]:
                            nc.vector.tensor_add(oah[:], oah[:], av[:])
                        nc.vector.tensor_tensor(
                            _q4h(oah[:]), _hb4h(rec[:]),
                            _q4h(oah[:]), op=ALU.mult)

                def o_tp_rt(rt):
                    for k in range(KT):
                        hf, kk = k // 8, k % 8
                        tp = ps_tp.tile([128, 128], ADT, tag="tp", name="tp")
                        nc.tensor.transpose(
                            tp[:], oa[rt][hf][:, kk * 128:(kk + 1) * 128],
                            ident_b[:])
                        nc.vector.tensor_copy(oT[rt][:, k, :], tp[:])

                def o_mm_rt(rt, po):
                    for ch in range(4):
                        po[ch] = ps_mm.tile([128, 512], F32, tag="mm",
                                            name=f"po{ch}")
                        for kg in range(4):
                            w = wts.tile([128, 4096], BF16, tag="w", name="w")
                            nc.sync.dma_start(w[:, :2048], o_d[:][ch * 4 + kg])
                            for i in range(4):
                                k = kg * 4 + i
                                nc.tensor.matmul(
                                    po[ch][:], oT[rt][:, k, :],
                                    w[:, i * 512:(i + 1) * 512],
                                    start=(k == 0), stop=(k == KT - 1))

                x2s = {}

                def norm2_mul(rt, o_parts):
                    rstd = rstd_of(rt, o_parts)
                    x = scr.tile([128, D], BF16, tag="scr", name=f"x2_{rt}")
                    nc.scalar.activation(x[:], h[rt][:], func=ACTF.Copy,
                                         scale=rstd[:])
                    x2s[rt] = x

                def norm2_tp(rt):
                    x = x2s[rt]
                    for k in range(KT):
                        tp = ps_tp.tile([128, 128], BF16, tag="tp", name="tp")
                        nc.tensor.transpose(
                            tp[:], x[:, k * 128:(k + 1) * 128], ident_b[:])
                        nc.vector.tensor_copy(
                            x2T[:, k, rt * 128:(rt + 1) * 128], tp[:])

                # ---- norm1 (raw transpose + rstd) + QKV, staggered ----
                for rt in range(RT):
                    rstds[rt] = rstd_of(rt, dn_parts)
                    if rt == 1 and t >= 1:
                        act_preload(ACTF.Exp)
                    tp_h(rt)
                    qkv_rt(rt, defer_copies=(rt == 1 and t >= 1))

                # ---- attention rt0 (overlaps QKV rt1 matmuls) ----
                attn_rt(0)
                # rt1's psum->SBUF copies, deferred so ACT never blocks exps(0)
                if t >= 1:
                    for ch in range(nch):
                        qkv_copy(1, ch, pq_held[ch])

                o_parts = [[] for _ in range(RT)]
                po0, po1 = {}, {}
                o_tp_rt(0)
                o_mm_rt(0, po0)
                # ---- attention rt1 (overlaps O-proj rt0 matmuls) ----
                attn_rt(1)
                for ch in range(4):
                    drain_add(0, ch, po0[ch], o_parts)
                norm2_mul(0, o_parts)
                o_tp_rt(1)
                o_mm_rt(1, po1)
                for ch in range(4):
                    drain_add(1, ch, po1[ch], o_parts)
                norm2_mul(1, o_parts)
                norm2_tp(0)
                norm2_tp(1)
                act_preload(ACTF.Sigmoid if SIM_SAFE else ACTF.Silu)

                # ---- MLP in two ff halves: gate/up -> mT, then down ----
                dn_parts = [[] for _ in range(RT)] if t < STEPS - 1 else None
                for half in range(2):
                    for pr in range(16 * half, 16 * (half + 1)):
                        wg = wts.tile([128, 4096], BF16, tag="w", name="wgt")
                        nc.sync.dma_start(wg[:], g_d[:][pr])
                        wu = wts.tile([128, 4096], BF16, tag="w", name="wut")
                        nc.sync.dma_start(wu[:], u_d[:][pr])
                        for mgi in range(2):
                            mloc = (pr * 2 + mgi) - 32 * half
                            pg = ps_gu.tile([128, R], F32, tag="gu", name="pg")
                            for k in range(KT):
                                c = (mgi * KT + k) * 128
                                nc.tensor.matmul(
                                    pg[:], wg[:, c:c + 128], x2T[:, k, :],
                                    start=(k == 0), stop=(k == KT - 1))
                            pu = ps_gu.tile([128, R], F32, tag="gu", name="pu")
                            for k in range(KT):
                                c = (mgi * KT + k) * 128
                                nc.tensor.matmul(
                                    pu[:], wu[:, c:c + 128], x2T[:, k, :],
                                    start=(k == 0), stop=(k == KT - 1))
                            sg = sm.tile([128, R], BF16, tag="sg", name="sg")
                            if SIM_SAFE:
                                # CoreSim lacks Silu; silu(x) = x * sigmoid(x)
                                nc.scalar.activation(sg[:], pg[:], func=ACTF.Sigmoid)
                                tmp = asc.tile([128, R], F32, tag="sgt", name="sgt")
                                nc.vector.tensor_tensor(
                                    tmp[:], sg[:], pg[:], op=ALU.mult)
                                nc.vector.tensor_tensor(
                                    mT[:, mloc, :], tmp[:], pu[:], op=ALU.mult)
                            else:
                                nc.scalar.activation(sg[:], pg[:], func=ACTF.Silu)
                                nc.vector.tensor_tensor(
                                    mT[:, mloc, :], sg[:], pu[:], op=ALU.mult)
                    for ch in range(4):
                        pd_ = [ps_mm.tile([128, 512], F32, tag="mm", name=f"pd{_rt}")
                               for _rt in range(RT)]
                        for kfg in range(4 * half, 4 * (half + 1)):
                            w = wts.tile([128, 4096], BF16, tag="w", name="wdt")
                            nc.sync.dma_start(w[:], d_d[:][ch * 8 + kfg])
                            for i in range(8):
                                kf = kfg * 8 + i
                                kfl = kf - 32 * half
                                for rt in range(RT):
                                    nc.tensor.matmul(
                                        pd_[rt][:],
                                        mT[:, kfl, rt * 128:(rt + 1) * 128],
                                        w[:, i * 512:(i + 1) * 512],
                                        start=(kf == 32 * half),
                                        stop=(kf == 32 * half + 31))
                        for rt in range(RT):
                            drain_add(rt, ch, pd_[rt],
                                      dn_parts if half == 1 else None)

                # ---- store step output ----
                for rt in range(RT):
                    nc.gpsimd.dma_start(out_d[:][t, rt], h[rt][:])

    nc.compile()
    return nc


# ---------------------------------------------------------------- host prep
def _rope_cs(t):
    inv = 1.0 / (THETA ** (np.arange(0, HD, 2, dtype=np.float64) / HD))
    emb = np.concatenate([t * inv, t * inv])
    return np.cos(emb), np.sin(emb)


def _rope_cols(w, t, nheads):
    # w: [D, nheads*HD] fp; returns rope'd version for position t
    w3 = w.reshape(D, nheads, HD)
    cos, sin = _rope_cs(t)
    wrot = np.concatenate([-w3[:, :, HD // 2:], w3[:, :, :HD // 2]], axis=2)
    return (w3 * cos[None, None, :] + wrot * sin[None, None, :]).reshape(D, nheads * HD)


def _pack_rhs(w, n_ch, n_kg):
    # w [K, n_ch*512]; chunks (ch, kg): [128, 4*512]; kg covers 4 k-tiles
    kt = w.shape[0] // 128
    A = w.reshape(n_kg, kt // n_kg, 128, n_ch, 512)
    return np.ascontiguousarray(A.transpose(3, 0, 2, 1, 4)).reshape(
        n_ch * n_kg, 128, (kt // n_kg) * 512)


def _pack_lhs_gu(w):
    # w [D, DFF] -> [32 pairs][128, (mgi 2, k 16, 128)]
    B = w.reshape(KT, 128, 32, 2, 128)
    return np.ascontiguousarray(B.transpose(2, 1, 3, 0, 4)).reshape(32, 128, 4096)


def _pack_rhs_dn(w):
    # w [DFF, D] -> chunks (ch 4, kfg 8): [128, (i 8, 512)]
    C = w.reshape(8, 8, 128, 4, 512)
    return np.ascontiguousarray(C.transpose(3, 0, 2, 1, 4)).reshape(32, 128, 4096)


def _gather_indices(comp_seq_lens, inst_lens):
    seqs = np.asarray(comp_seq_lens)
    insts = np.asarray(inst_lens)
    idx, off = [], 0
    for s, i in zip(seqs, insts):
        s, i = int(s), int(i)
        idx.append(np.arange(off + i - 1, off + s - 1))
        off += s
    return np.concatenate(idx)


def _prep_inputs(hidden_states, comp_seq_lens, inst_lens, w_q, w_k, w_v, w_o,
                 ln1_w, ln2_w, w_gate, w_up, w_down):
    idx = _gather_indices(comp_seq_lens, inst_lens)
    h0 = np.asarray(hidden_states, np.float32)[0, idx]          # [N, D]
    N = h0.shape[0]
    assert N == NCORES * R, f"expected {NCORES*R} rows, got {N}"

    ln1 = np.asarray(ln1_w, np.float64)
    ln2 = np.asarray(ln2_w, np.float64)
    wq_e = np.asarray(w_q, np.float64) * ln1[:, None] * (HD ** -0.5)
    wk_e = np.asarray(w_k, np.float64) * ln1[:, None]
    wv_e = np.asarray(w_v, np.float64) * ln1[:, None]
    wg_e = np.asarray(w_gate, np.float64) * ln2[:, None]
    wu_e = np.asarray(w_up, np.float64) * ln2[:, None]

    qkv_pack = np.empty((STEPS, 24, 128, 2048), NP_W)
    for t in range(STEPS):
        wq_t = _rope_cols(wq_e, t, HEADS)
        wk_t = _rope_cols(wk_e, t, KVH)
        qkv = np.concatenate([wk_t, wv_e, wq_t], axis=1)  # K, V, Q order
        qkv_pack[t] = _pack_rhs(qkv, 6, 4).astype(NP_W)

    weights = {
        "wqkv": qkv_pack,
        "wo": _pack_rhs(np.asarray(w_o, np.float64), 4, 4).astype(NP_W),
        "wg": _pack_lhs_gu(wg_e).astype(NP_W),
        "wu": _pack_lhs_gu(wu_e).astype(NP_W),
        "wd": _pack_rhs_dn(np.asarray(w_down, np.float64)).astype(NP_W),
    }
    h0_cores = h0.reshape(NCORES, RT, 128, D)
    return weights, h0_cores


def kernel(**inputs):
    weights, h0_cores = _prep_inputs(**inputs)

    if "nc" not in _CACHE:
        _CACHE["nc"] = _build_program()
    nc = _CACHE["nc"]

    in_maps = [dict(weights, h0=np.ascontiguousarray(h0_cores[c]))
               for c in range(NCORES)]
    res = run_bass_kernel_spmd(nc, in_maps, core_ids=list(range(NCORES)))
    _CACHE["last_results"] = res

    outs = []
    for c in range(NCORES):
        o = res.results[c]["out"]                  # [5, RT, 128, D]
        outs.append(o.reshape(STEPS, R, D).transpose(1, 0, 2))
    return np.concatenate(outs, axis=0)            # [N, 5, D]
